# revision 12
# baseline (speedup 1.0000x reference)
"""LSQ quantizer forward kernel for Trainium2 (8 NeuronCores, data-parallel).

Computes out = (round(clip((x @ H) / s, -Qn, Qp)) * s) @ H.T  for
x [4, 4096, 2048] f32, H [2048, 2048] f32, by sharding the 16384 token rows
across 8 cores (2048 rows each).

Math notes:
- s = (scale - scale*gs) + scale*gs with gs = 1/sqrt(num_elements*Qp), computed
  in fp32 on host exactly as the reference does (stop_gradient is a no-op in
  the forward value).
- x @ H runs on the PE array as a 3-term bf16 split (xh@Hh + xh@Hl + xl@Hh)
  which carries fp32-grade precision (~4e-6 rel) at bf16 speed.
- round() is the magic-number trick ((v + 1.5*2^23) - 1.5*2^23), which is
  round-half-to-even, identical to jnp.round.
- q = clip(round(x_rot/s)) takes integer values in [-Qn, Qp], exact in bf16,
  so q @ H.T needs only a 2-term split on H.T (q@Th + q@Tl).
- The first matmul is computed transposed (psum = (x@H)^T tile) so that the
  quantized result q^T is directly the stationary operand layout for the
  second matmul, whose PSUM output lands in natural [row, col] orientation.
"""
import numpy as np
import ml_dtypes
from contextlib import ExitStack

import concourse.bacc as bacc
import concourse.mybir as mybir
import concourse.tile as tile
from concourse.bass_utils import run_bass_kernel_spmd

BF16 = mybir.dt.bfloat16
F32 = mybir.dt.float32

N_CORES = 8
D = 2048                 # feature dim
ROWS_TOTAL = 4 * 4096    # B * S
M_ROWS = ROWS_TOTAL // N_CORES   # rows per core (2048)

KT = D // 128            # 16 contraction tiles
NT = D // 128            # 16 output-column tiles (phase 1)
MAGIC = float(np.float32(3 << 22))   # 1.5 * 2^23


def _build_kernel(inv_s: float, s: float, qn: float, qp: float):
    nc = bacc.Bacc(trn_type="TRN2")

    xh_d = nc.dram_tensor("xh", [M_ROWS, D], BF16, kind="ExternalInput")
    xl_d = nc.dram_tensor("xl", [M_ROWS, D], BF16, kind="ExternalInput")
    hh_d = nc.dram_tensor("hh", [D, D], BF16, kind="ExternalInput")
    hl_d = nc.dram_tensor("hl", [D, D], BF16, kind="ExternalInput")
    th_d = nc.dram_tensor("th", [D, D], BF16, kind="ExternalInput")
    tl_d = nc.dram_tensor("tl", [D, D], BF16, kind="ExternalInput")
    out_d = nc.dram_tensor("out", [M_ROWS, D], F32, kind="ExternalOutput")

    n_strips = M_ROWS // 512

    with tile.TileContext(nc) as tc:
        with ExitStack() as ctx:
            dram_pool = ctx.enter_context(
                tc.tile_pool(name="dram", bufs=1, space="DRAM")
            )
            q_dram = dram_pool.tile([D, M_ROWS], BF16)   # q^T layout [n, m]
            ps_pool = ctx.enter_context(
                tc.tile_pool(name="ps", bufs=4, space="PSUM")
            )

            # ---------------- Phase 1: q^T = quant((x @ H)^T) ----------------
            with (
                tc.tile_pool(name="hmat", bufs=KT) as hpool,
                tc.tile_pool(name="xt", bufs=18) as xtpool,
                tc.tile_pool(name="uv", bufs=3) as uvpool,
                tc.tile_pool(name="qo", bufs=4) as qpool,
            ):
                hh_sb = []
                hl_sb = []
                for k in range(KT):
                    t1 = hpool.tile([128, D], BF16, tag="hh")
                    nc.sync.dma_start(t1[:], hh_d[k * 128:(k + 1) * 128, :])
                    hh_sb.append(t1)
                    t2 = hpool.tile([128, D], BF16, tag="hl")
                    nc.sync.dma_start(t2[:], hl_d[k * 128:(k + 1) * 128, :])
                    hl_sb.append(t2)

                for st in range(n_strips):
                    m0 = st * 512
                    xth = []
                    xtl = []
                    for k in range(KT):
                        t1 = xtpool.tile([128, 512], BF16, tag="xth")
                        nc.sync.dma_start(
                            t1[:], xh_d[m0:m0 + 512, k * 128:(k + 1) * 128],
                            transpose=True,
                        )
                        xth.append(t1)
                        t2 = xtpool.tile([128, 512], BF16, tag="xtl")
                        nc.sync.dma_start(
                            t2[:], xl_d[m0:m0 + 512, k * 128:(k + 1) * 128],
                            transpose=True,
                        )
                        xtl.append(t2)

                    for n in range(NT):
                        ps1 = ps_pool.tile([128, 512], F32, tag="ps")
                        nsl = slice(n * 128, (n + 1) * 128)
                        for k in range(KT):
                            first = k == 0
                            last = k == KT - 1
                            nc.tensor.matmul(
                                ps1[:], hh_sb[k][:, nsl], xth[k][:],
                                start=first, stop=False,
                            )
                            nc.tensor.matmul(
                                ps1[:], hl_sb[k][:, nsl], xth[k][:],
                                start=False, stop=False,
                            )
                            nc.tensor.matmul(
                                ps1[:], hh_sb[k][:, nsl], xtl[k][:],
                                start=False, stop=last,
                            )
                        # quantize: q = max(min((ps*inv_s + MAGIC) - MAGIC, qp), -qn)
                        u = uvpool.tile([128, 512], F32, tag="u")
                        nc.vector.tensor_scalar(
                            out=u[:], in0=ps1[:],
                            scalar1=inv_s, scalar2=MAGIC,
                            op0=mybir.AluOpType.mult, op1=mybir.AluOpType.add,
                        )
                        v = uvpool.tile([128, 512], F32, tag="v")
                        nc.vector.tensor_scalar(
                            out=v[:], in0=u[:],
                            scalar1=MAGIC, scalar2=qp,
                            op0=mybir.AluOpType.subtract, op1=mybir.AluOpType.min,
                        )
                        q = qpool.tile([128, 512], BF16, tag="q")
                        nc.vector.tensor_scalar_max(
                            out=q[:], in0=v[:], scalar1=-qn
                        )
                        nc.sync.dma_start(
                            q_dram[n * 128:(n + 1) * 128, m0:m0 + 512], q[:]
                        )

            # ---------------- Phase 2: out = (q @ H^T) * s ----------------
            with (
                tc.tile_pool(name="tmat", bufs=KT) as tpool,
                tc.tile_pool(name="qi", bufs=18) as qipool,
                tc.tile_pool(name="op", bufs=4) as opool,
            ):
                th_sb = []
                tl_sb = []
                for n in range(KT):
                    t1 = tpool.tile([128, D], BF16, tag="th")
                    nc.sync.dma_start(t1[:], th_d[n * 128:(n + 1) * 128, :])
                    th_sb.append(t1)
                    t2 = tpool.tile([128, D], BF16, tag="tl")
                    nc.sync.dma_start(t2[:], tl_d[n * 128:(n + 1) * 128, :])
                    tl_sb.append(t2)

                for st in range(n_strips):
                    m0 = st * 512
                    qs = []
                    for n in range(KT):
                        t1 = qipool.tile([128, 512], BF16, tag="qs")
                        nc.sync.dma_start(
                            t1[:], q_dram[n * 128:(n + 1) * 128, m0:m0 + 512]
                        )
                        qs.append(t1)

                    for ms in range(4):         # 128-row output tiles in strip
                        msl = slice(ms * 128, (ms + 1) * 128)
                        for j in range(4):      # 512-col output blocks
                            jsl = slice(j * 512, (j + 1) * 512)
                            ps2 = ps_pool.tile([128, 512], F32, tag="ps")
                            for n in range(KT):
                                nc.tensor.matmul(
                                    ps2[:], qs[n][:, msl], th_sb[n][:, jsl],
                                    start=(n == 0), stop=False,
                                )
                                nc.tensor.matmul(
                                    ps2[:], qs[n][:, msl], tl_sb[n][:, jsl],
                                    start=False, stop=(n == KT - 1),
                                )
                            o = opool.tile([128, 512], F32, tag="o")
                            nc.scalar.mul(o[:], ps2[:], s)
                            nc.sync.dma_start(
                                out_d[m0 + ms * 128:m0 + (ms + 1) * 128, jsl],
                                o[:],
                            )

    nc.finalize()
    return nc


def _split_bf16(a):
    hi = a.astype(ml_dtypes.bfloat16)
    lo = (a - hi.astype(np.float32)).astype(ml_dtypes.bfloat16)
    return np.ascontiguousarray(hi), np.ascontiguousarray(lo)


_CACHE = {}


def kernel(x, scale, hadamard, Qn, Qp, num_elements):
    x = np.asarray(x, dtype=np.float32)
    h = np.asarray(hadamard, dtype=np.float32)
    scale_f = np.float32(np.asarray(scale).reshape(-1)[0])
    qn = float(np.asarray(Qn))
    qp = float(np.asarray(Qp))
    ne = float(np.asarray(num_elements))

    # forward value of s, replicating the reference's fp32 op order
    gs = np.float32(1.0) / np.sqrt(np.float32(ne) * np.float32(qp))
    bw = scale_f * gs
    s = (scale_f - bw) + bw
    inv_s = np.float32(1.0) / s

    key = (float(s), qn, qp)
    if key not in _CACHE:
        _CACHE[key] = _build_kernel(float(inv_s), float(s), qn, qp)
    nc = _CACHE[key]

    hh, hl = _split_bf16(h)
    th, tl = _split_bf16(np.ascontiguousarray(h.T))

    xf = x.reshape(ROWS_TOTAL, D)
    in_maps = []
    for c in range(N_CORES):
        xs = xf[c * M_ROWS:(c + 1) * M_ROWS]
        xsh, xsl = _split_bf16(xs)
        in_maps.append(
            {"xh": xsh, "xl": xsl, "hh": hh, "hl": hl, "th": th, "tl": tl}
        )

    res = run_bass_kernel_spmd(nc, in_maps, core_ids=list(range(N_CORES)))
    out = np.concatenate([res.results[c]["out"] for c in range(N_CORES)], axis=0)
    return out.reshape(x.shape)


def _prep_in_maps(x, hadamard):
    h = np.asarray(hadamard, dtype=np.float32)
    hh, hl = _split_bf16(h)
    th, tl = _split_bf16(np.ascontiguousarray(h.T))
    xf = np.asarray(x, dtype=np.float32).reshape(ROWS_TOTAL, D)
    in_maps = []
    for c in range(N_CORES):
        xsh, xsl = _split_bf16(xf[c * M_ROWS:(c + 1) * M_ROWS])
        in_maps.append(
            {"xh": xsh, "xl": xsl, "hh": hh, "hl": hl, "th": th, "tl": tl}
        )
    return in_maps


def profile_once(inputs):
    """Rerun with NTFF tracing, return HW exec time in ns (or None)."""
    nc = next(iter(_CACHE.values()))
    try:
        res = run_bass_kernel_spmd(
            nc, _prep_in_maps(inputs["x"], inputs["hadamard"]),
            core_ids=list(range(N_CORES)), trace=True,
        )
        if res.exec_time_ns is not None:
            return res.exec_time_ns
    except Exception:
        pass
    # NTFF hook unavailable under this axon client: fall back to the
    # InstructionCostModel device-occupancy timeline (per-core, SPMD).
    from concourse.timeline_sim import TimelineSim

    return int(TimelineSim(nc, trace=False).simulate())
